# revision 1
# baseline (speedup 1.0000x reference)
"""Trainium2 Bass kernel for DeepseekV3 naive MoE (expert-parallel over 8 cores).

Contract: kernel(**inputs) takes FULL unsharded numpy inputs
(hidden_states [T,H] f32, top_k_index [T,K] i32, top_k_weights [T,K] f32,
wg [E,H,I] f32, wu [E,H,I] f32, wd [E,I,H] f32) and returns the FULL
[T,H] f32 output, equal to the reference grouped-GEMM MoE.

Strategy (hardcoded for T=8192, H=1024, I=1408, E=32, K=8, 8 cores):
 - Host: replicate tokens K times, stable-sort pairs by expert id, pad each
   expert's group to a shared capacity cap = ceil128(max group size) (the
   device program is compiled and cached per cap), build transposed
   activations xT [H, cap] per expert (contraction dim on partitions), cast
   matmul operands to bf16 (PSUM accumulation stays fp32).
 - Device (SPMD, 4 experts per core): for each expert, for each 512-column
   chunk: gateT = wg.T @ xT, upT = wu.T @ xT (PSUM f32 accumulate over H),
   actT = silu(gateT) * upT (bf16), downT = wd.T @ actT, scaled by the
   per-pair router weight, stored as [H, CAP] per expert.
 - Host: transpose back, unsort, sum the K weighted contributions per token.
"""

import os
import sys

for _p in ("/opt/trn_rl_repo", "/root/.axon_site/_ro/trn_rl_repo"):
    if _p not in sys.path:
        sys.path.insert(0, _p)

# recover wedged NeuronCores on first touch; NEFF cache across rounds
os.environ.setdefault("NEURON_RT_RESET_CORES", "1")
os.environ.setdefault("MYCRO_LOCAL_CACHE", "1")

import numpy as np
import ml_dtypes

import concourse.bass as bass  # noqa: F401  (registers types)
import concourse.tile as tile
from concourse import bacc, mybir

# Problem dims (fixed by the task)
E, H, I, K, T = 32, 1024, 1408, 8, 8192
N_CORES = 8
EL = E // N_CORES  # experts per core
P = 128
HO, IO = H // P, I // P  # 8, 11
CHUNK = 512
CAP_QUANTUM = 128


def _chunks_of(cap: int):
    out = []
    off = 0
    while off < cap:
        w = min(CHUNK, cap - off)
        out.append((off, w))
        off += w
    return out

BF16 = ml_dtypes.bfloat16

_CACHE: dict = {}


def _build_nc(cap: int, repeat: int = 1):
    """Build + compile the per-core Bass kernel for per-expert capacity `cap`.

    repeat>1 duplicates the whole schedule in-kernel (same IO); used only to
    amortize launch overhead when measuring device execution time."""
    assert cap % CAP_QUANTUM == 0
    chunk_list = _chunks_of(cap)
    dt_mm = mybir.dt.bfloat16

    nc = bacc.Bacc("TRN2", target_bir_lowering=False, debug=False)

    xT_d = nc.dram_tensor("xT", [EL, HO, P, cap], dt_mm, kind="ExternalInput")
    wg_d = nc.dram_tensor("wg", [EL, HO, P, I], dt_mm, kind="ExternalInput")
    wu_d = nc.dram_tensor("wu", [EL, HO, P, I], dt_mm, kind="ExternalInput")
    wd_d = nc.dram_tensor("wd", [EL, IO, P, H], dt_mm, kind="ExternalInput")
    wr_d = nc.dram_tensor("wr", [EL, P, cap], mybir.dt.float32, kind="ExternalInput")
    out_d = nc.dram_tensor("out", [EL, HO, P, cap], mybir.dt.float32, kind="ExternalOutput")

    sched = [(e, off, w) for e in range(EL) for (off, w) in chunk_list] * repeat

    with tile.TileContext(nc) as tc:
        with (
            tc.tile_pool(name="wpool", bufs=2) as wpool,
            tc.tile_pool(name="wdpool", bufs=2) as wdpool,
            tc.tile_pool(name="xpool", bufs=3) as xpool,
            tc.tile_pool(name="apool", bufs=2) as apool,
            tc.tile_pool(name="opool", bufs=3) as opool,
            tc.tile_pool(name="rpool", bufs=2) as rpool,
            tc.tile_pool(name="gps", bufs=2, space="PSUM") as gps,
            tc.tile_pool(name="ups", bufs=2, space="PSUM") as ups,
            tc.tile_pool(name="dps", bufs=3, space="PSUM") as dps,
        ):
            wtiles = {}  # live weight tiles for current expert
            act_tiles = {}  # chunk index -> act tile
            x_live = {}

            def emit_gu(j):
                e, off, w = sched[j]
                if off == 0:
                    wgt = wpool.tile([P, HO, I], dt_mm, tag="wg")
                    for ho in range(HO):
                        nc.sync.dma_start(wgt[:, ho, :], wg_d[e, ho])
                    wut = wpool.tile([P, HO, I], dt_mm, tag="wu")
                    for ho in range(HO):
                        nc.sync.dma_start(wut[:, ho, :], wu_d[e, ho])
                    wdt = wdpool.tile([P, IO, H], dt_mm, tag="wd")
                    for io in range(IO):
                        nc.sync.dma_start(wdt[:, io, :], wd_d[e, io])
                    wrt = rpool.tile([P, cap], mybir.dt.float32, tag="wr")
                    nc.sync.dma_start(wrt[:], wr_d[e])
                    wtiles[e] = (wgt, wut, wdt, wrt)
                wgt, wut, wdt, wrt = wtiles[e]
                xt = xpool.tile([P, HO, w], dt_mm, tag="x")
                nc.sync.dma_start(
                    xt[:], xT_d[e, :, :, off : off + w].rearrange("h p n -> p h n")
                )
                x_live[j] = xt
                at = apool.tile([P, IO, w], dt_mm, tag="act")
                act_tiles[j] = at
                for it in range(IO):
                    g_ps = gps.tile([P, w], mybir.dt.float32, tag="g")
                    u_ps = ups.tile([P, w], mybir.dt.float32, tag="u")
                    for ho in range(HO):
                        nc.tensor.matmul(
                            g_ps[:],
                            wgt[:, ho, it * P : (it + 1) * P],
                            xt[:, ho, :],
                            start=(ho == 0),
                            stop=(ho == HO - 1),
                        )
                    for ho in range(HO):
                        nc.tensor.matmul(
                            u_ps[:],
                            wut[:, ho, it * P : (it + 1) * P],
                            xt[:, ho, :],
                            start=(ho == 0),
                            stop=(ho == HO - 1),
                        )
                    nc.scalar.activation(
                        at[:, it, :], g_ps[:], mybir.ActivationFunctionType.Silu
                    )
                    nc.vector.tensor_mul(at[:, it, :], at[:, it, :], u_ps[:])

            def emit_down(j):
                e, off, w = sched[j]
                _, _, wdt, wrt = wtiles[e]
                at = act_tiles.pop(j)
                for ht in range(HO):
                    d_ps = dps.tile([P, w], mybir.dt.float32, tag="d")
                    for it in range(IO):
                        nc.tensor.matmul(
                            d_ps[:],
                            wdt[:, it, ht * P : (ht + 1) * P],
                            at[:, it, :],
                            start=(it == 0),
                            stop=(it == IO - 1),
                        )
                    ot = opool.tile([P, w], mybir.dt.float32, tag="o")
                    nc.vector.tensor_mul(ot[:], d_ps[:], wrt[:, off : off + w])
                    nc.sync.dma_start(out_d[e, ht, :, off : off + w], ot[:])
                del x_live[j]

            for j in range(len(sched) + 1):
                if j < len(sched):
                    emit_gu(j)
                if j >= 1:
                    emit_down(j - 1)

    nc.compile()
    return nc


def _get_nc(cap: int, repeat: int = 1):
    key = ("nc", cap, repeat)
    if key not in _CACHE:
        _CACHE[key] = _build_nc(cap, repeat)
    return _CACHE[key]


def _get_runner(cap: int, repeat: int = 1):
    """Cached jitted SPMD executor for the kernel (avoids re-tracing per call).

    Mirrors bass2jax.run_bass_via_pjrt's multi-core path, but without output
    donation: this kernel writes every output element, so the result buffers
    don't need to be pre-zeroed, and a non-donating executable can be invoked
    repeatedly on device-resident inputs for timing.
    """
    key = ("runner", cap, repeat)
    if key in _CACHE:
        return _CACHE[key]

    import jax
    from jax.sharding import Mesh, PartitionSpec
    from jax.experimental.shard_map import shard_map
    from concourse import bass2jax, mybir as _mybir

    nc = _get_nc(cap, repeat)
    bass2jax.install_neuronx_cc_hook()

    partition_name = nc.partition_id_tensor.name if nc.partition_id_tensor else None
    in_names, out_names, out_avals, zero_outs = [], [], [], []
    for alloc in nc.m.functions[0].allocations:
        if not isinstance(alloc, _mybir.MemoryLocationSet):
            continue
        name = alloc.memorylocations[0].name
        if alloc.kind == "ExternalInput":
            if name != partition_name:
                in_names.append(name)
        elif alloc.kind == "ExternalOutput":
            out_names.append(name)
            shape = tuple(alloc.tensor_shape)
            dtype = _mybir.dt.np(alloc.dtype)
            out_avals.append(jax.core.ShapedArray(shape, dtype))
            zero_outs.append(np.zeros(shape, dtype))
    n_params = len(in_names)
    all_names = in_names + out_names
    if partition_name is not None:
        all_names = all_names + [partition_name]

    def _body(*args):
        operands = list(args)
        if partition_name is not None:
            operands.append(bass2jax.partition_id_tensor())
        outs = bass2jax._bass_exec_p.bind(
            *operands,
            out_avals=tuple(out_avals),
            in_names=tuple(all_names),
            out_names=tuple(out_names),
            lowering_input_output_aliases=(),
            sim_require_finite=True,
            sim_require_nnan=True,
            nc=nc,
        )
        return tuple(outs)

    devices = jax.devices()[:N_CORES]
    mesh = Mesh(np.asarray(devices), ("core",))
    n_all = n_params + len(out_names)
    sharded = jax.jit(
        shard_map(
            _body,
            mesh=mesh,
            in_specs=(PartitionSpec("core"),) * n_all,
            out_specs=(PartitionSpec("core"),) * len(out_names),
            check_rep=False,
        ),
        keep_unused=True,
    )
    runner = {
        "fn": sharded,
        "in_names": in_names,
        "out_names": out_names,
        "out_avals": out_avals,
        "zero_outs": zero_outs,
    }
    _CACHE[key] = runner
    return runner


def _run_spmd(cap: int, in_maps):
    r = _get_runner(cap)
    concat_in = [
        np.concatenate([np.asarray(m[name]) for m in in_maps], axis=0)
        for name in r["in_names"]
    ]
    concat_zero = [
        np.zeros((N_CORES * z.shape[0], *z.shape[1:]), z.dtype) for z in r["zero_outs"]
    ]
    out_arrs = r["fn"](*concat_in, *concat_zero)
    return [
        {
            name: np.asarray(out_arrs[i]).reshape(N_CORES, *r["out_avals"][i].shape)[c]
            for i, name in enumerate(r["out_names"])
        }
        for c in range(N_CORES)
    ]


def _dispatch(hidden_states, top_k_index, top_k_weights, wg, wu, wd):
    """Host-side routing: sort pairs by expert, pad per-expert groups, build
    per-core input maps. Returns (cap, in_maps, sort_idx, offsets)."""
    hidden_states = np.ascontiguousarray(hidden_states, dtype=np.float32)
    flat_eid = np.asarray(top_k_index, dtype=np.int64).ravel()
    sort_idx = np.argsort(flat_eid, kind="stable")
    tok = sort_idx // K
    counts = np.bincount(flat_eid, minlength=E)
    offsets = np.concatenate(([0], np.cumsum(counts)))

    # smallest 128-multiple capacity covering the largest expert group; the
    # device program is compiled (and cached) per cap value
    cap = max(CHUNK, int(-(-int(counts.max()) // CAP_QUANTUM) * CAP_QUANTUM))

    # sorted, weighted dispatch tensors
    xs_T = np.ascontiguousarray(hidden_states[tok].T)  # [H, T*K] sorted by expert
    w_sorted = np.asarray(top_k_weights, dtype=np.float32).ravel()[sort_idx]

    in_maps = []
    for core in range(N_CORES):
        xT = np.zeros((EL, H, cap), dtype=BF16)
        wr = np.zeros((EL, P, cap), dtype=np.float32)
        for le in range(EL):
            e = core * EL + le
            o0, o1 = offsets[e], offsets[e + 1]
            g = o1 - o0
            xT[le, :, :g] = xs_T[:, o0:o1]
            wr[le, :, :g] = w_sorted[o0:o1][None, :]
        es = slice(core * EL, (core + 1) * EL)
        in_maps.append(
            {
                "xT": xT.reshape(EL, HO, P, cap),
                "wg": _cast_cached(wg, es, (EL, HO, P, I)),
                "wu": _cast_cached(wu, es, (EL, HO, P, I)),
                "wd": _cast_cached(wd, es, (EL, IO, P, H)),
                "wr": wr,
            }
        )
    return cap, in_maps, sort_idx, offsets


def _cast_cached(w, es, shape):
    # bf16 cast of a weight slice, cached on the source array identity (id +
    # cheap fingerprint) so reused weight tensors across calls skip the cast
    w = np.asarray(w)
    fp = (id(w), w.shape, float(w.flat[0]), float(w.flat[w.size // 2 + 1]),
          float(w.flat[w.size - 1]), es.start, es.stop)
    key = ("wcast", fp, shape)
    if key not in _CACHE:
        _CACHE[key] = np.ascontiguousarray(w[es].reshape(shape)).astype(BF16)
    return _CACHE[key]


def kernel(hidden_states, top_k_index, top_k_weights, wg, wu, wd):
    Tn, Hn = hidden_states.shape
    En, _, In = wg.shape
    Kn = top_k_index.shape[1]
    assert (Tn, Hn, En, In, Kn) == (T, H, E, I, K), "kernel hardcoded for spec shapes"

    cap, in_maps, sort_idx, offsets = _dispatch(
        hidden_states, top_k_index, top_k_weights, wg, wu, wd
    )
    results = _run_spmd(cap, in_maps)

    # combine: weighted contributions are already applied on device
    down_sorted = np.empty((T * K, H), dtype=np.float32)
    for core in range(N_CORES):
        o = results[core]["out"].reshape(EL, H, cap)
        for le in range(EL):
            e = core * EL + le
            o0, o1 = offsets[e], offsets[e + 1]
            down_sorted[o0:o1] = o[le, :, : o1 - o0].T

    inv = np.empty(T * K, dtype=np.int64)
    inv[sort_idx] = np.arange(T * K)
    out = down_sorted[inv].reshape(T, K, H).sum(axis=1, dtype=np.float32)
    return out.astype(np.float32)


def measure_hw_ns(inputs, n_rep=5, repeat=5):
    """Amortized per-execution device time (ns): difference between a kernel
    variant that runs the whole schedule `repeat` times in one NEFF and the
    1x kernel, divided by (repeat-1). Launch overhead (~80ms under axon)
    cancels in the difference."""
    import time
    import jax
    from jax.sharding import Mesh, NamedSharding, PartitionSpec

    cap, in_maps, _, _ = _dispatch(**inputs)

    mesh = Mesh(np.asarray(jax.devices()[:N_CORES]), ("core",))
    sh = NamedSharding(mesh, PartitionSpec("core"))

    def timed(rep):
        r = _get_runner(cap, rep)
        concat_in = [
            np.concatenate([np.asarray(m[name]) for m in in_maps], axis=0)
            for name in r["in_names"]
        ]
        concat_zero = [
            np.zeros((N_CORES * z.shape[0], *z.shape[1:]), z.dtype)
            for z in r["zero_outs"]
        ]
        dev_in = [jax.device_put(a, sh) for a in concat_in]
        dev_zero = [jax.device_put(a, sh) for a in concat_zero]
        jax.block_until_ready(r["fn"](*dev_in, *dev_zero))  # warm/compile
        ts = []
        for _ in range(n_rep):
            t0 = time.perf_counter()
            jax.block_until_ready(r["fn"](*dev_in, *dev_zero))
            ts.append(time.perf_counter() - t0)
        return min(ts)

    # interleaved rounds so session drift (thermal/terminal load) cancels
    timed(1)
    timed(repeat)
    slopes = []
    for _ in range(3):
        t1 = timed(1)
        tk = timed(repeat)
        slopes.append((tk - t1) / (repeat - 1) * 1e9)
    slopes.sort()
    return slopes[len(slopes) // 2]



# revision 6
# speedup vs baseline: 1.4327x; 1.4327x over previous
"""Trainium2 Bass kernel for DeepseekV3 naive MoE (expert-parallel over 8 cores).

Contract: kernel(**inputs) takes FULL unsharded numpy inputs
(hidden_states [T,H] f32, top_k_index [T,K] i32, top_k_weights [T,K] f32,
wg [E,H,I] f32, wu [E,H,I] f32, wd [E,I,H] f32) and returns the FULL
[T,H] f32 output, equal to the reference grouped-GEMM MoE.

Strategy (hardcoded for T=8192, H=1024, I=1408, E=32, K=8, 8 cores):
 - Host: replicate tokens K times, stable-sort pairs by expert id. Sort the
   32 experts by group size and deal them round-robin into 4 slots x 8 cores,
   so the 8 experts sharing a slot have near-identical group sizes. Each
   slot's width = max group size in the slot (rounded to 8); the device
   program is compiled and cached per width-tuple. Build transposed
   activations xT [HO,P,W] (contraction dim on partitions), cast matmul
   operands to bf16 (PSUM accumulation stays fp32).
 - Device (SPMD, one expert per slot per core): for each slot, for each
   ~width/ceil(width/512)-column chunk: gateT = wg.T @ xT, upT = wu.T @ xT
   (PSUM f32 accumulate over H), actT = silu(gateT) * upT (bf16),
   downT = wd.T @ actT, scaled by the per-pair router weight, stored as
   [HO,P,W].
 - Host: transpose back, unsort, sum the K weighted contributions per token.
"""

import os
import sys

for _p in ("/opt/trn_rl_repo", "/root/.axon_site/_ro/trn_rl_repo"):
    if _p not in sys.path:
        sys.path.insert(0, _p)

# recover wedged NeuronCores on first touch; NEFF cache across rounds
os.environ.setdefault("NEURON_RT_RESET_CORES", "1")
os.environ.setdefault("MYCRO_LOCAL_CACHE", "1")

import numpy as np
import ml_dtypes

import concourse.bass as bass  # noqa: F401  (registers types)
import concourse.tile as tile
from concourse import bacc, mybir

# Problem dims (fixed by the task)
E, H, I, K, T = 32, 1024, 1408, 8, 8192
N_CORES = 8
EL = E // N_CORES  # experts (slots) per core
P = 128
HO, IO = H // P, I // P  # 8, 11
CHUNK = 512

BF16 = ml_dtypes.bfloat16

_CACHE: dict = {}


def _chunks_of(width: int):
    """Split a slot width into ceil(width/CHUNK) near-equal chunks (<=512)."""
    n = -(-width // CHUNK)
    base, rem = divmod(width, n)
    out, off = [], 0
    for i in range(n):
        w = base + (1 if i < rem else 0)
        out.append((off, w))
        off += w
    return out


def _build_nc(widths: tuple, repeat: int = 1):
    """Build + compile the per-core Bass kernel for slot widths `widths`.

    repeat>1 duplicates the whole schedule in-kernel (same IO); used only to
    amortize launch overhead when measuring device execution time."""
    assert len(widths) == EL
    W = sum(widths)
    bases = [sum(widths[:j]) for j in range(EL)]
    dt_mm = mybir.dt.bfloat16

    nc = bacc.Bacc("TRN2", target_bir_lowering=False, debug=False)

    xT_d = nc.dram_tensor("xT", [HO, P, W], dt_mm, kind="ExternalInput")
    # wg/wu are it-major: [slot, it, p, ho*128+n] with element value
    # wg[slot][h=ho*P+p, i=it*128+n]; one [P, H]-contiguous DMA per it-tile
    wg_d = nc.dram_tensor("wg", [EL, IO, P, H], dt_mm, kind="ExternalInput")
    wu_d = nc.dram_tensor("wu", [EL, IO, P, H], dt_mm, kind="ExternalInput")
    wd_d = nc.dram_tensor("wd", [EL, IO, P, H], dt_mm, kind="ExternalInput")
    wr_d = nc.dram_tensor("wr", [P, W], mybir.dt.float32, kind="ExternalInput")
    out_d = nc.dram_tensor("out", [HO, P, W], mybir.dt.float32, kind="ExternalOutput")

    # entry: (slot, col, width, is_first_chunk_of_slot, prefetch_slot_or_None)
    # prefetch_slot: issue the NEXT slot's weight DMAs while this chunk's
    # compute still has ~2 chunks of runway, hiding the ~24us load.
    one_rep = []
    for j in range(EL):
        chunks = _chunks_of(widths[j])
        for ci, (off, w) in enumerate(chunks):
            pf = (j + 1) % EL if ci == max(len(chunks) - 2, 0) else None
            one_rep.append((j, bases[j] + off, w, ci == 0, pf))
    sched = one_rep * repeat
    # the final slot of the final rep has nothing to prefetch
    tail = len(_chunks_of(widths[EL - 1]))
    sched[-tail:] = [(j0, o0, w0, f0, None) for (j0, o0, w0, f0, _) in sched[-tail:]]

    with tile.TileContext(nc) as tc:
        with (
            tc.tile_pool(name="wpool", bufs=2) as wpool,
            tc.tile_pool(name="wdpool", bufs=2) as wdpool,
            tc.tile_pool(name="xpool", bufs=3) as xpool,
            tc.tile_pool(name="apool", bufs=2) as apool,
            tc.tile_pool(name="opool", bufs=3) as opool,
            tc.tile_pool(name="rpool", bufs=2) as rpool,
            tc.tile_pool(name="gps", bufs=2, space="PSUM") as gps,
            tc.tile_pool(name="ups", bufs=2, space="PSUM") as ups,
            tc.tile_pool(name="dps", bufs=3, space="PSUM") as dps,
        ):
            wtiles = {}  # live weight tiles per slot (overwritten as reps wrap)
            act_tiles = {}  # schedule index -> act tile
            x_live = {}

            def emit_weights_gu(j, startup_x=None):
                # per-it tiles so the first gate/up matmuls only wait on the
                # first ~0.5MB of weights, not the whole 5.8MB load
                wg_its, wu_its = [], []
                for it in range(IO):
                    wgt = wpool.tile([P, HO, P], dt_mm, tag=f"wg{it}")
                    nc.sync.dma_start(wgt[:].rearrange("p h n -> p (h n)"), wg_d[j, it])
                    wut = wpool.tile([P, HO, P], dt_mm, tag=f"wu{it}")
                    nc.sync.dma_start(wut[:].rearrange("p h n -> p (h n)"), wu_d[j, it])
                    wg_its.append(wgt)
                    wu_its.append(wut)
                wtiles[j] = [wg_its, wu_its, None, None]

            def emit_weights_d(j):
                # wr/wd are first needed by the slot's first down, ~1.5 chunks
                # after gate/up start — keep them off the startup critical path
                wrt = rpool.tile([P, widths[j]], mybir.dt.float32, tag="wr")
                nc.sync.dma_start(wrt[:], wr_d[:, bases[j] : bases[j] + widths[j]])
                wdt = wdpool.tile([P, IO, H], dt_mm, tag="wd")
                for io in range(IO):
                    nc.sync.dma_start(wdt[:, io, :], wd_d[j, io])
                wtiles[j][2] = wdt
                wtiles[j][3] = wrt

            pre_x = {}

            def emit_gu(idx):
                j, o, w, first, pf = sched[idx]
                wg_its, wu_its, wdt, wrt = wtiles[j]
                xt = pre_x.pop(idx, None)
                if xt is None:
                    xt = xpool.tile([P, HO, w], dt_mm, tag="x")
                    nc.sync.dma_start(
                        xt[:], xT_d[:, :, o : o + w].rearrange("h p n -> p h n")
                    )
                x_live[idx] = xt
                at = apool.tile([P, IO, w], dt_mm, tag="act")
                act_tiles[idx] = at
                for it in range(IO):
                    g_ps = gps.tile([P, w], mybir.dt.float32, tag="g")
                    u_ps = ups.tile([P, w], mybir.dt.float32, tag="u")
                    for ho in range(HO):
                        nc.tensor.matmul(
                            g_ps[:],
                            wg_its[it][:, ho, :],
                            xt[:, ho, :],
                            start=(ho == 0),
                            stop=(ho == HO - 1),
                        )
                    for ho in range(HO):
                        nc.tensor.matmul(
                            u_ps[:],
                            wu_its[it][:, ho, :],
                            xt[:, ho, :],
                            start=(ho == 0),
                            stop=(ho == HO - 1),
                        )
                    nc.scalar.activation(
                        at[:, it, :], g_ps[:], mybir.ActivationFunctionType.Silu
                    )
                    nc.vector.tensor_mul(at[:, it, :], at[:, it, :], u_ps[:])

            def emit_down(idx):
                j, o, w, _, _ = sched[idx]
                _, _, wdt, wrt = wtiles[j]
                at = act_tiles.pop(idx)
                roff = o - bases[j]
                for ht in range(HO):
                    d_ps = dps.tile([P, w], mybir.dt.float32, tag="d")
                    for it in range(IO):
                        nc.tensor.matmul(
                            d_ps[:],
                            wdt[:, it, ht * P : (ht + 1) * P],
                            at[:, it, :],
                            start=(it == 0),
                            stop=(it == IO - 1),
                        )
                    ot = opool.tile([P, w], mybir.dt.float32, tag="o")
                    nc.vector.tensor_mul(ot[:], d_ps[:], wrt[:, roff : roff + w])
                    nc.sync.dma_start(out_d[ht, :, o : o + w], ot[:])
                del x_live[idx]

            # startup fill: x(chunk 0) right after the first wg/wu it-tile,
            # so the first gate matmul starts ~3us in
            j0, o0, w0, _, _ = sched[0]

            xt0 = xpool.tile([P, HO, w0], dt_mm, tag="x")
            nc.sync.dma_start(
                xt0[:], xT_d[:, :, o0 : o0 + w0].rearrange("h p n -> p h n")
            )
            pre_x[0] = xt0
            emit_weights_gu(j0)
            for idx in range(len(sched) + 1):
                if idx < len(sched):
                    j, o, w, first, pf = sched[idx]
                    if first and idx >= 1:
                        # slot boundary: keep PE fed with the previous slot's
                        # final down while the new slot's weights land
                        emit_down(idx - 1)
                        emit_gu(idx)
                    else:
                        emit_gu(idx)
                        if idx >= 1:
                            emit_down(idx - 1)
                    if idx == 0:
                        emit_weights_d(0)  # wd after the first x chunk
                    if pf is not None:
                        emit_weights_gu(pf)
                        emit_weights_d(pf)
                else:
                    emit_down(idx - 1)

    nc.compile()
    return nc


def _get_nc(widths: tuple, repeat: int = 1):
    key = ("nc", widths, repeat)
    if key not in _CACHE:
        _CACHE[key] = _build_nc(widths, repeat)
    return _CACHE[key]


def _get_runner(widths: tuple, repeat: int = 1):
    """Cached jitted SPMD executor for the kernel (avoids re-tracing per call).

    Mirrors bass2jax.run_bass_via_pjrt's multi-core path, but without output
    donation: this kernel writes every output element, so the result buffers
    don't need to be pre-zeroed, and a non-donating executable can be invoked
    repeatedly on device-resident inputs for timing.
    """
    key = ("runner", widths, repeat)
    if key in _CACHE:
        return _CACHE[key]

    import jax
    from jax.sharding import Mesh, PartitionSpec
    from jax.experimental.shard_map import shard_map
    from concourse import bass2jax, mybir as _mybir

    nc = _get_nc(widths, repeat)
    bass2jax.install_neuronx_cc_hook()

    partition_name = nc.partition_id_tensor.name if nc.partition_id_tensor else None
    in_names, out_names, out_avals, zero_outs = [], [], [], []
    for alloc in nc.m.functions[0].allocations:
        if not isinstance(alloc, _mybir.MemoryLocationSet):
            continue
        name = alloc.memorylocations[0].name
        if alloc.kind == "ExternalInput":
            if name != partition_name:
                in_names.append(name)
        elif alloc.kind == "ExternalOutput":
            out_names.append(name)
            shape = tuple(alloc.tensor_shape)
            dtype = _mybir.dt.np(alloc.dtype)
            out_avals.append(jax.core.ShapedArray(shape, dtype))
            zero_outs.append(np.zeros(shape, dtype))
    n_params = len(in_names)
    all_names = in_names + out_names
    if partition_name is not None:
        all_names = all_names + [partition_name]

    def _body(*args):
        operands = list(args)
        if partition_name is not None:
            operands.append(bass2jax.partition_id_tensor())
        outs = bass2jax._bass_exec_p.bind(
            *operands,
            out_avals=tuple(out_avals),
            in_names=tuple(all_names),
            out_names=tuple(out_names),
            lowering_input_output_aliases=(),
            sim_require_finite=True,
            sim_require_nnan=True,
            nc=nc,
        )
        return tuple(outs)

    devices = jax.devices()[:N_CORES]
    mesh = Mesh(np.asarray(devices), ("core",))
    n_all = n_params + len(out_names)
    sharded = jax.jit(
        shard_map(
            _body,
            mesh=mesh,
            in_specs=(PartitionSpec("core"),) * n_all,
            out_specs=(PartitionSpec("core"),) * len(out_names),
            check_rep=False,
        ),
        keep_unused=True,
    )
    runner = {
        "fn": sharded,
        "in_names": in_names,
        "out_names": out_names,
        "out_avals": out_avals,
        "zero_outs": zero_outs,
    }
    _CACHE[key] = runner
    return runner


def _run_spmd(widths: tuple, in_maps):
    r = _get_runner(widths)
    concat_in = [
        np.concatenate([np.asarray(m[name]) for m in in_maps], axis=0)
        for name in r["in_names"]
    ]
    concat_zero = [
        np.zeros((N_CORES * z.shape[0], *z.shape[1:]), z.dtype) for z in r["zero_outs"]
    ]
    out_arrs = r["fn"](*concat_in, *concat_zero)
    return [
        {
            name: np.asarray(out_arrs[i]).reshape(N_CORES, *r["out_avals"][i].shape)[c]
            for i, name in enumerate(r["out_names"])
        }
        for c in range(N_CORES)
    ]


def _assign(counts):
    """Deal experts (sorted by group size, descending) round-robin into
    EL slots x N_CORES cores; slot width = max size in slot, rounded to 8."""
    order = np.argsort(-counts, kind="stable")
    slots = order.reshape(EL, N_CORES)  # [slot, core] -> expert id
    widths = tuple(int(-(-int(counts[slots[j]].max()) // 8) * 8) for j in range(EL))
    return slots, widths


def _dispatch(hidden_states, top_k_index, top_k_weights, wg, wu, wd):
    """Host-side routing: sort pairs by expert, assign experts to balanced
    slots, build per-core input maps. Returns (widths, in_maps, combine_info)."""
    hidden_states = np.ascontiguousarray(hidden_states, dtype=np.float32)
    flat_eid = np.asarray(top_k_index, dtype=np.int64).ravel()
    sort_idx = np.argsort(flat_eid, kind="stable")
    tok = sort_idx // K
    counts = np.bincount(flat_eid, minlength=E)
    offsets = np.concatenate(([0], np.cumsum(counts)))
    slots, widths = _assign(counts)
    W = sum(widths)
    bases = [sum(widths[:j]) for j in range(EL)]

    # sorted, weighted dispatch tensors
    xs_T = np.ascontiguousarray(hidden_states[tok].T)  # [H, T*K] sorted by expert
    w_sorted = np.asarray(top_k_weights, dtype=np.float32).ravel()[sort_idx]

    in_maps = []
    for core in range(N_CORES):
        xT = np.zeros((H, W), dtype=BF16)
        wr = np.zeros((P, W), dtype=np.float32)
        eids = []
        for j in range(EL):
            e = int(slots[j][core])
            eids.append(e)
            o0, o1 = offsets[e], offsets[e + 1]
            g = o1 - o0
            b = bases[j]
            xT[:, b : b + g] = xs_T[:, o0:o1]
            wr[:, b : b + g] = w_sorted[o0:o1][None, :]
        eids = tuple(eids)
        in_maps.append(
            {
                "xT": xT.reshape(HO, P, W),
                "wg": _cast_cached(wg, eids, "it_major"),
                "wu": _cast_cached(wu, eids, "it_major"),
                "wd": _cast_cached(wd, eids, "k_major"),
                "wr": wr,
            }
        )
    return widths, in_maps, (slots, bases, offsets, sort_idx)


def _cast_cached(w, eids, layout):
    # bf16 cast of a weight slice, cached on the source array identity (id +
    # cheap fingerprint) so reused weight tensors across calls skip the cast
    w = np.asarray(w)
    fp = (id(w), w.shape, float(w.flat[0]), float(w.flat[w.size // 2 + 1]),
          float(w.flat[w.size - 1]), eids)
    key = ("wcast", fp, layout)
    if key not in _CACHE:
        ws = w[list(eids)]  # [EL, K_dim, N_dim]
        kd, nd = ws.shape[1], ws.shape[2]
        if layout == "k_major":
            # [EL, KO, P, N]: tile partition = contraction dim
            out = ws.reshape(EL, kd // P, P, nd)
        else:
            # it_major: [EL, NO, P, KO*P] with [e, it, p, ko*P+n] =
            # ws[e, ko*P+p, it*P+n] — per-it-tile contiguous [P, K] rows
            out = ws.reshape(EL, kd // P, P, nd // P, P).transpose(0, 3, 2, 1, 4)
            out = out.reshape(EL, nd // P, P, kd)
        _CACHE[key] = np.ascontiguousarray(out).astype(BF16)
    return _CACHE[key]


def kernel(hidden_states, top_k_index, top_k_weights, wg, wu, wd):
    Tn, Hn = hidden_states.shape
    En, _, In = wg.shape
    Kn = top_k_index.shape[1]
    assert (Tn, Hn, En, In, Kn) == (T, H, E, I, K), "kernel hardcoded for spec shapes"

    widths, in_maps, (slots, bases, offsets, sort_idx) = _dispatch(
        hidden_states, top_k_index, top_k_weights, wg, wu, wd
    )
    results = _run_spmd(widths, in_maps)

    # combine: weighted contributions are already applied on device
    down_sorted = np.empty((T * K, H), dtype=np.float32)
    for core in range(N_CORES):
        o = results[core]["out"].reshape(H, sum(widths))
        for j in range(EL):
            e = int(slots[j][core])
            o0, o1 = offsets[e], offsets[e + 1]
            b = bases[j]
            down_sorted[o0:o1] = o[:, b : b + (o1 - o0)].T

    inv = np.empty(T * K, dtype=np.int64)
    inv[sort_idx] = np.arange(T * K)
    out = down_sorted[inv].reshape(T, K, H).sum(axis=1, dtype=np.float32)
    return out.astype(np.float32)


def measure_hw_ns(inputs, n_rep=5, repeat=5):
    """Amortized per-execution device time (ns): difference between a kernel
    variant that runs the whole schedule `repeat` times in one NEFF and the
    1x kernel, divided by (repeat-1). Launch overhead (~80ms under axon)
    cancels in the difference."""
    import time
    import jax
    from jax.sharding import Mesh, NamedSharding, PartitionSpec

    widths, in_maps, _ = _dispatch(**inputs)

    mesh = Mesh(np.asarray(jax.devices()[:N_CORES]), ("core",))
    sh = NamedSharding(mesh, PartitionSpec("core"))

    def timed(rep):
        r = _get_runner(widths, rep)
        concat_in = [
            np.concatenate([np.asarray(m[name]) for m in in_maps], axis=0)
            for name in r["in_names"]
        ]
        concat_zero = [
            np.zeros((N_CORES * z.shape[0], *z.shape[1:]), z.dtype)
            for z in r["zero_outs"]
        ]
        dev_in = [jax.device_put(a, sh) for a in concat_in]
        dev_zero = [jax.device_put(a, sh) for a in concat_zero]
        jax.block_until_ready(r["fn"](*dev_in, *dev_zero))  # warm/compile
        ts = []
        for _ in range(n_rep):
            t0 = time.perf_counter()
            jax.block_until_ready(r["fn"](*dev_in, *dev_zero))
            ts.append(time.perf_counter() - t0)
        return min(ts)

    # interleaved rounds so session drift (thermal/terminal load) cancels
    timed(1)
    timed(repeat)
    slopes = []
    for _ in range(7):
        t1 = timed(1)
        tk = timed(repeat)
        slopes.append((tk - t1) / (repeat - 1) * 1e9)
    slopes.sort()
    med = slopes[len(slopes) // 2]
    if med > 0:
        return med
    # launch-latency noise can exceed the signal on a loaded box; a
    # non-positive marginal rep time is unphysical, so fall back to the
    # median of the positive slopes
    pos = [s for s in slopes if s > 0]
    return pos[len(pos) // 2] if pos else med


# revision 7
# speedup vs baseline: 2.8393x; 1.9818x over previous
"""Trainium2 Bass kernel for DeepseekV3 naive MoE (expert-parallel over 8 cores).

Contract: kernel(**inputs) takes FULL unsharded numpy inputs
(hidden_states [T,H] f32, top_k_index [T,K] i32, top_k_weights [T,K] f32,
wg [E,H,I] f32, wu [E,H,I] f32, wd [E,I,H] f32) and returns the FULL
[T,H] f32 output, equal to the reference grouped-GEMM MoE.

Strategy (hardcoded for T=8192, H=1024, I=1408, E=32, K=8, 8 cores):
 - Host: replicate tokens K times, stable-sort pairs by expert id. Sort the
   32 experts by group size and deal them round-robin into 4 slots x 8 cores,
   so the 8 experts sharing a slot have near-identical group sizes. Each
   slot's width = max group size in the slot (rounded to 8); the device
   program is compiled and cached per width-tuple. Build transposed
   activations xT [HO,P,W] (contraction dim on partitions), cast matmul
   operands to bf16 (PSUM accumulation stays fp32).
 - Device (SPMD, one expert per slot per core): for each slot, for each
   ~width/ceil(width/512)-column chunk: gateT = wg.T @ xT, upT = wu.T @ xT
   (PSUM f32 accumulate over H), actT = silu(gateT) * upT (bf16),
   downT = wd.T @ actT, scaled by the per-pair router weight, stored as
   [HO,P,W].
 - Host: transpose back, unsort, sum the K weighted contributions per token.
"""

import os
import sys

for _p in ("/opt/trn_rl_repo", "/root/.axon_site/_ro/trn_rl_repo"):
    if _p not in sys.path:
        sys.path.insert(0, _p)

# recover wedged NeuronCores on first touch; NEFF cache across rounds
os.environ.setdefault("NEURON_RT_RESET_CORES", "1")
os.environ.setdefault("MYCRO_LOCAL_CACHE", "1")

import numpy as np
import ml_dtypes

import concourse.bass as bass  # noqa: F401  (registers types)
import concourse.tile as tile
from concourse import bacc, mybir

# Problem dims (fixed by the task)
E, H, I, K, T = 32, 1024, 1408, 8, 8192
N_CORES = 8
EL = E // N_CORES  # experts (slots) per core
P = 128
HO, IO = H // P, I // P  # 8, 11
CHUNK = 512

BF16 = ml_dtypes.bfloat16

_CACHE: dict = {}


def _chunks_of(width: int):
    """Split a slot width into ceil(width/CHUNK) near-equal chunks (<=512)."""
    n = -(-width // CHUNK)
    base, rem = divmod(width, n)
    out, off = [], 0
    for i in range(n):
        w = base + (1 if i < rem else 0)
        out.append((off, w))
        off += w
    return out


def _build_nc(widths: tuple, repeat: int = 1):
    """Build + compile the per-core Bass kernel for slot widths `widths`.

    repeat>1 duplicates the whole schedule in-kernel (same IO); used only to
    amortize launch overhead when measuring device execution time."""
    assert len(widths) == EL
    W = sum(widths)
    bases = [sum(widths[:j]) for j in range(EL)]
    dt_mm = mybir.dt.bfloat16

    nc = bacc.Bacc("TRN2", target_bir_lowering=False, debug=False)

    xT_d = nc.dram_tensor("xT", [HO, P, W], dt_mm, kind="ExternalInput")
    # wg/wu are it-major: [slot, it, p, ho*128+n] with element value
    # wg[slot][h=ho*P+p, i=it*128+n]; one [P, H]-contiguous DMA per it-tile
    wg_d = nc.dram_tensor("wg", [EL, IO, P, H], dt_mm, kind="ExternalInput")
    wu_d = nc.dram_tensor("wu", [EL, IO, P, H], dt_mm, kind="ExternalInput")
    wd_d = nc.dram_tensor("wd", [EL, IO, P, H], dt_mm, kind="ExternalInput")
    wr_d = nc.dram_tensor("wr", [P, W], mybir.dt.float32, kind="ExternalInput")
    out_d = nc.dram_tensor("out", [HO, P, W], mybir.dt.float32, kind="ExternalOutput")

    # entry: (slot, col, width, is_first_chunk_of_slot, prefetch_slot_or_None)
    # prefetch_slot: issue the NEXT slot's weight DMAs while this chunk's
    # compute still has ~2 chunks of runway, hiding the ~24us load.
    one_rep = []
    for j in range(EL):
        chunks = _chunks_of(widths[j])
        for ci, (off, w) in enumerate(chunks):
            pf = (j + 1) % EL if ci == max(len(chunks) - 2, 0) else None
            one_rep.append((j, bases[j] + off, w, ci == 0, pf))
    sched = one_rep * repeat
    # the final slot of the final rep has nothing to prefetch
    tail = len(_chunks_of(widths[EL - 1]))
    sched[-tail:] = [(j0, o0, w0, f0, None) for (j0, o0, w0, f0, _) in sched[-tail:]]

    with tile.TileContext(nc) as tc:
        with (
            tc.tile_pool(name="wpool", bufs=2) as wpool,
            tc.tile_pool(name="wdpool", bufs=2) as wdpool,
            tc.tile_pool(name="xpool", bufs=3) as xpool,
            tc.tile_pool(name="apool", bufs=2) as apool,
            tc.tile_pool(name="opool", bufs=3) as opool,
            tc.tile_pool(name="rpool", bufs=2) as rpool,
            tc.tile_pool(name="gps", bufs=2, space="PSUM") as gps,
            tc.tile_pool(name="ups", bufs=2, space="PSUM") as ups,
            tc.tile_pool(name="dps", bufs=3, space="PSUM") as dps,
        ):
            wtiles = {}  # live weight tiles per slot (overwritten as reps wrap)
            act_tiles = {}  # schedule index -> act tile
            x_live = {}

            def emit_weights_gu(j, startup_x=None):
                # per-it tiles so the first gate/up matmuls only wait on the
                # first ~0.5MB of weights, not the whole 5.8MB load
                wg_its, wu_its = [], []
                for it in range(IO):
                    wgt = wpool.tile([P, HO, P], dt_mm, tag=f"wg{it}")
                    nc.sync.dma_start(wgt[:].rearrange("p h n -> p (h n)"), wg_d[j, it])
                    wut = wpool.tile([P, HO, P], dt_mm, tag=f"wu{it}")
                    nc.sync.dma_start(wut[:].rearrange("p h n -> p (h n)"), wu_d[j, it])
                    wg_its.append(wgt)
                    wu_its.append(wut)
                wtiles[j] = [wg_its, wu_its, None, None]

            def emit_weights_d(j):
                # wr/wd are first needed by the slot's first down, ~1.5 chunks
                # after gate/up start — keep them off the startup critical path
                wrt = rpool.tile([P, widths[j]], mybir.dt.float32, tag="wr")
                nc.sync.dma_start(wrt[:], wr_d[:, bases[j] : bases[j] + widths[j]])
                wdt = wdpool.tile([P, IO, H], dt_mm, tag="wd")
                for io in range(IO):
                    nc.sync.dma_start(wdt[:, io, :], wd_d[j, io])
                wtiles[j][2] = wdt
                wtiles[j][3] = wrt

            pre_x = {}

            def emit_gu(idx):
                j, o, w, first, pf = sched[idx]
                wg_its, wu_its, wdt, wrt = wtiles[j]
                xt = pre_x.pop(idx, None)
                if xt is None:
                    xt = xpool.tile([P, HO, w], dt_mm, tag="x")
                    nc.sync.dma_start(
                        xt[:], xT_d[:, :, o : o + w].rearrange("h p n -> p h n")
                    )
                x_live[idx] = xt
                at = apool.tile([P, IO, w], dt_mm, tag="act")
                act_tiles[idx] = at
                for it in range(IO):
                    g_ps = gps.tile([P, w], mybir.dt.float32, tag="g")
                    u_ps = ups.tile([P, w], mybir.dt.float32, tag="u")
                    for ho in range(HO):
                        nc.tensor.matmul(
                            g_ps[:],
                            wg_its[it][:, ho, :],
                            xt[:, ho, :],
                            start=(ho == 0),
                            stop=(ho == HO - 1),
                        )
                    for ho in range(HO):
                        nc.tensor.matmul(
                            u_ps[:],
                            wu_its[it][:, ho, :],
                            xt[:, ho, :],
                            start=(ho == 0),
                            stop=(ho == HO - 1),
                        )
                    nc.scalar.activation(
                        at[:, it, :], g_ps[:], mybir.ActivationFunctionType.Silu
                    )
                    nc.vector.tensor_mul(at[:, it, :], at[:, it, :], u_ps[:])

            def emit_down(idx):
                j, o, w, _, _ = sched[idx]
                _, _, wdt, wrt = wtiles[j]
                at = act_tiles.pop(idx)
                roff = o - bases[j]
                for ht in range(HO):
                    d_ps = dps.tile([P, w], mybir.dt.float32, tag="d")
                    for it in range(IO):
                        nc.tensor.matmul(
                            d_ps[:],
                            wdt[:, it, ht * P : (ht + 1) * P],
                            at[:, it, :],
                            start=(it == 0),
                            stop=(it == IO - 1),
                        )
                    ot = opool.tile([P, w], mybir.dt.float32, tag="o")
                    nc.vector.tensor_mul(ot[:], d_ps[:], wrt[:, roff : roff + w])
                    nc.sync.dma_start(out_d[ht, :, o : o + w], ot[:])
                del x_live[idx]

            # startup fill: x(chunk 0) right after the first wg/wu it-tile,
            # so the first gate matmul starts ~3us in
            j0, o0, w0, _, _ = sched[0]

            xt0 = xpool.tile([P, HO, w0], dt_mm, tag="x")
            nc.sync.dma_start(
                xt0[:], xT_d[:, :, o0 : o0 + w0].rearrange("h p n -> p h n")
            )
            pre_x[0] = xt0
            emit_weights_gu(j0)
            for idx in range(len(sched) + 1):
                if idx < len(sched):
                    j, o, w, first, pf = sched[idx]
                    if first and idx >= 1:
                        # slot boundary: keep PE fed with the previous slot's
                        # final down while the new slot's weights land
                        emit_down(idx - 1)
                        emit_gu(idx)
                    else:
                        emit_gu(idx)
                        if idx >= 1:
                            emit_down(idx - 1)
                    if idx == 0:
                        emit_weights_d(0)  # wd after the first x chunk
                    if pf is not None:
                        emit_weights_gu(pf)
                        emit_weights_d(pf)
                else:
                    emit_down(idx - 1)

    nc.compile()
    return nc


def _get_nc(widths: tuple, repeat: int = 1):
    key = ("nc", widths, repeat)
    if key not in _CACHE:
        _CACHE[key] = _build_nc(widths, repeat)
    return _CACHE[key]


def _get_runner(widths: tuple, repeat: int = 1):
    """Cached jitted SPMD executor for the kernel (avoids re-tracing per call).

    Mirrors bass2jax.run_bass_via_pjrt's multi-core path, but without output
    donation: this kernel writes every output element, so the result buffers
    don't need to be pre-zeroed, and a non-donating executable can be invoked
    repeatedly on device-resident inputs for timing.
    """
    key = ("runner", widths, repeat)
    if key in _CACHE:
        return _CACHE[key]

    import jax
    from jax.sharding import Mesh, PartitionSpec
    from jax.experimental.shard_map import shard_map
    from concourse import bass2jax, mybir as _mybir

    nc = _get_nc(widths, repeat)
    bass2jax.install_neuronx_cc_hook()

    partition_name = nc.partition_id_tensor.name if nc.partition_id_tensor else None
    in_names, out_names, out_avals, zero_outs = [], [], [], []
    for alloc in nc.m.functions[0].allocations:
        if not isinstance(alloc, _mybir.MemoryLocationSet):
            continue
        name = alloc.memorylocations[0].name
        if alloc.kind == "ExternalInput":
            if name != partition_name:
                in_names.append(name)
        elif alloc.kind == "ExternalOutput":
            out_names.append(name)
            shape = tuple(alloc.tensor_shape)
            dtype = _mybir.dt.np(alloc.dtype)
            out_avals.append(jax.core.ShapedArray(shape, dtype))
            zero_outs.append(np.zeros(shape, dtype))
    n_params = len(in_names)
    all_names = in_names + out_names
    if partition_name is not None:
        all_names = all_names + [partition_name]

    def _body(*args):
        operands = list(args)
        if partition_name is not None:
            operands.append(bass2jax.partition_id_tensor())
        outs = bass2jax._bass_exec_p.bind(
            *operands,
            out_avals=tuple(out_avals),
            in_names=tuple(all_names),
            out_names=tuple(out_names),
            lowering_input_output_aliases=(),
            sim_require_finite=True,
            sim_require_nnan=True,
            nc=nc,
        )
        return tuple(outs)

    devices = jax.devices()[:N_CORES]
    mesh = Mesh(np.asarray(devices), ("core",))
    n_all = n_params + len(out_names)
    sharded = jax.jit(
        shard_map(
            _body,
            mesh=mesh,
            in_specs=(PartitionSpec("core"),) * n_all,
            out_specs=(PartitionSpec("core"),) * len(out_names),
            check_rep=False,
        ),
        keep_unused=True,
    )
    runner = {
        "fn": sharded,
        "in_names": in_names,
        "out_names": out_names,
        "out_avals": out_avals,
        "zero_outs": zero_outs,
    }
    _CACHE[key] = runner
    return runner


def _run_spmd(widths: tuple, in_maps):
    r = _get_runner(widths)
    concat_in = [
        np.concatenate([np.asarray(m[name]) for m in in_maps], axis=0)
        for name in r["in_names"]
    ]
    concat_zero = [
        np.zeros((N_CORES * z.shape[0], *z.shape[1:]), z.dtype) for z in r["zero_outs"]
    ]
    out_arrs = r["fn"](*concat_in, *concat_zero)
    return [
        {
            name: np.asarray(out_arrs[i]).reshape(N_CORES, *r["out_avals"][i].shape)[c]
            for i, name in enumerate(r["out_names"])
        }
        for c in range(N_CORES)
    ]


def _assign(counts):
    """Deal experts (sorted by group size, descending) round-robin into
    EL slots x N_CORES cores; slot width = max size in slot, rounded to 8."""
    order = np.argsort(-counts, kind="stable")
    slots = order.reshape(EL, N_CORES)  # [slot, core] -> expert id
    widths = tuple(int(-(-int(counts[slots[j]].max()) // 4) * 4) for j in range(EL))
    return slots, widths


def _dispatch(hidden_states, top_k_index, top_k_weights, wg, wu, wd):
    """Host-side routing: sort pairs by expert, assign experts to balanced
    slots, build per-core input maps. Returns (widths, in_maps, combine_info)."""
    hidden_states = np.ascontiguousarray(hidden_states, dtype=np.float32)
    flat_eid = np.asarray(top_k_index, dtype=np.int64).ravel()
    sort_idx = np.argsort(flat_eid, kind="stable")
    tok = sort_idx // K
    counts = np.bincount(flat_eid, minlength=E)
    offsets = np.concatenate(([0], np.cumsum(counts)))
    slots, widths = _assign(counts)
    W = sum(widths)
    bases = [sum(widths[:j]) for j in range(EL)]

    # sorted, weighted dispatch tensors
    xs_T = np.ascontiguousarray(hidden_states[tok].T)  # [H, T*K] sorted by expert
    w_sorted = np.asarray(top_k_weights, dtype=np.float32).ravel()[sort_idx]

    in_maps = []
    for core in range(N_CORES):
        xT = np.zeros((H, W), dtype=BF16)
        wr = np.zeros((P, W), dtype=np.float32)
        eids = []
        for j in range(EL):
            e = int(slots[j][core])
            eids.append(e)
            o0, o1 = offsets[e], offsets[e + 1]
            g = o1 - o0
            b = bases[j]
            xT[:, b : b + g] = xs_T[:, o0:o1]
            wr[:, b : b + g] = w_sorted[o0:o1][None, :]
        eids = tuple(eids)
        in_maps.append(
            {
                "xT": xT.reshape(HO, P, W),
                "wg": _cast_cached(wg, eids, "it_major"),
                "wu": _cast_cached(wu, eids, "it_major"),
                "wd": _cast_cached(wd, eids, "k_major"),
                "wr": wr,
            }
        )
    return widths, in_maps, (slots, bases, offsets, sort_idx)


def _cast_cached(w, eids, layout):
    # bf16 cast of a weight slice, cached on the source array identity (id +
    # cheap fingerprint) so reused weight tensors across calls skip the cast
    w = np.asarray(w)
    fp = (id(w), w.shape, float(w.flat[0]), float(w.flat[w.size // 2 + 1]),
          float(w.flat[w.size - 1]), eids)
    key = ("wcast", fp, layout)
    if key not in _CACHE:
        ws = w[list(eids)]  # [EL, K_dim, N_dim]
        kd, nd = ws.shape[1], ws.shape[2]
        if layout == "k_major":
            # [EL, KO, P, N]: tile partition = contraction dim
            out = ws.reshape(EL, kd // P, P, nd)
        else:
            # it_major: [EL, NO, P, KO*P] with [e, it, p, ko*P+n] =
            # ws[e, ko*P+p, it*P+n] — per-it-tile contiguous [P, K] rows
            out = ws.reshape(EL, kd // P, P, nd // P, P).transpose(0, 3, 2, 1, 4)
            out = out.reshape(EL, nd // P, P, kd)
        _CACHE[key] = np.ascontiguousarray(out).astype(BF16)
    return _CACHE[key]


def kernel(hidden_states, top_k_index, top_k_weights, wg, wu, wd):
    Tn, Hn = hidden_states.shape
    En, _, In = wg.shape
    Kn = top_k_index.shape[1]
    assert (Tn, Hn, En, In, Kn) == (T, H, E, I, K), "kernel hardcoded for spec shapes"

    widths, in_maps, (slots, bases, offsets, sort_idx) = _dispatch(
        hidden_states, top_k_index, top_k_weights, wg, wu, wd
    )
    results = _run_spmd(widths, in_maps)

    # combine: weighted contributions are already applied on device
    down_sorted = np.empty((T * K, H), dtype=np.float32)
    for core in range(N_CORES):
        o = results[core]["out"].reshape(H, sum(widths))
        for j in range(EL):
            e = int(slots[j][core])
            o0, o1 = offsets[e], offsets[e + 1]
            b = bases[j]
            down_sorted[o0:o1] = o[:, b : b + (o1 - o0)].T

    inv = np.empty(T * K, dtype=np.int64)
    inv[sort_idx] = np.arange(T * K)
    out = down_sorted[inv].reshape(T, K, H).sum(axis=1, dtype=np.float32)
    return out.astype(np.float32)


def measure_hw_ns(inputs, n_rep=5, repeat=5):
    """Amortized per-execution device time (ns): difference between a kernel
    variant that runs the whole schedule `repeat` times in one NEFF and the
    1x kernel, divided by (repeat-1). Launch overhead (~80ms under axon)
    cancels in the difference."""
    import time
    import jax
    from jax.sharding import Mesh, NamedSharding, PartitionSpec

    widths, in_maps, _ = _dispatch(**inputs)

    mesh = Mesh(np.asarray(jax.devices()[:N_CORES]), ("core",))
    sh = NamedSharding(mesh, PartitionSpec("core"))

    def timed(rep):
        r = _get_runner(widths, rep)
        concat_in = [
            np.concatenate([np.asarray(m[name]) for m in in_maps], axis=0)
            for name in r["in_names"]
        ]
        concat_zero = [
            np.zeros((N_CORES * z.shape[0], *z.shape[1:]), z.dtype)
            for z in r["zero_outs"]
        ]
        dev_in = [jax.device_put(a, sh) for a in concat_in]
        dev_zero = [jax.device_put(a, sh) for a in concat_zero]
        jax.block_until_ready(r["fn"](*dev_in, *dev_zero))  # warm/compile
        ts = []
        for _ in range(n_rep):
            t0 = time.perf_counter()
            jax.block_until_ready(r["fn"](*dev_in, *dev_zero))
            ts.append(time.perf_counter() - t0)
        return min(ts)

    # interleaved rounds so session drift (thermal/terminal load) cancels
    timed(1)
    timed(repeat)
    slopes = []
    for _ in range(7):
        t1 = timed(1)
        tk = timed(repeat)
        slopes.append((tk - t1) / (repeat - 1) * 1e9)
    slopes.sort()
    med = slopes[len(slopes) // 2]
    if med > 0:
        return med
    # launch-latency noise can exceed the signal on a loaded box; a
    # non-positive marginal rep time is unphysical, so fall back to the
    # median of the positive slopes
    pos = [s for s in slopes if s > 0]
    return pos[len(pos) // 2] if pos else med
